# revision 13
# baseline (speedup 1.0000x reference)
"""Trainium2 Bass kernel v6 for nn_MaskedSelfAttention (B=8, L=2048, DX=1024, DA=2048).

Data-parallel over B (one batch per core). s = x (wq wk^T) x^T via C-trick;
column softmax (done row-wise in transposed space); out = p * v^T.

v6 vs v5b: single-pass f32r everywhere (no h/l split pairs), mean-centering
(u,w,c) + exact column-sum (S) rank-1 reconstruction gives the precision
(emulated rel err 6.7e-3 vs gate 2e-2; v5b measured 5.8e-3):
    C  = R(wq) R(wk)^T          1-pass f32r      [DX,DX] fp32 in SBUF
    dC = C - (u-c) - w          spilled f32 to DRAM (PE rounds at read)
    v  = R(x) R(wv)             1-pass f32r, bf16 spill
    yd = dC^T R(x)^T + (w-c) x S_row   1-pass, stored f32r in SBUF
    sT = R(x) yd + S[j]*a_row[i]; a_row = c*S_row + x@du
    S_row EXACT from x natural layout via DVE free-axis reduce (this exactness
    cancels the coherent part of x's f32r rounding; everything else is sloppy)
    mask applied exactly via zm = (z1+1000)*m - 1000 (the -1000 folds into the
    exp bias, so masked entries get exactly -1000 as in the reference)
Phase order C -> V -> Y -> J keeps PE dense; x-stats stream during V.
"""
import sys
sys.path.insert(0, "/opt/trn_rl_repo")
import numpy as np

import concourse.bacc as bacc
import concourse.tile as tile
import concourse.mybir as mybir
from concourse.bass_utils import run_bass_kernel_spmd

dt = mybir.dt
F32 = dt.float32
F32R = dt.float32r
U8 = dt.uint8
BF16 = dt.bfloat16
I32 = dt.int32
AF = mybir.ActivationFunctionType
ALU = mybir.AluOpType
AX = mybir.AxisListType

B, L, DX, DA = 8, 2048, 1024, 2048
P = 128
NTL = L // P      # 16
NTX = DX // P     # 8
NTA = DA // P     # 16
CH = 512
SCALE = float(1.0 / np.sqrt(np.float32(DA)))


def build():
    nc = bacc.Bacc("TRN2", target_bir_lowering=False, debug=False, num_devices=8)

    xT_d = nc.declare_dram_parameter("xT", [DX, L], F32R, isOutput=False)
    xN_d = nc.declare_dram_parameter("xN", [L, DX], F32, isOutput=False)
    wqT_d = nc.declare_dram_parameter("wqT", [DA, DX], F32R, isOutput=False)
    wkT_d = nc.declare_dram_parameter("wkT", [DA, DX], F32R, isOutput=False)
    wv_d = nc.declare_dram_parameter("wv", [DX, DA], F32R, isOutput=False)
    maskT_d = nc.declare_dram_parameter("maskT", [L, L], U8, isOutput=False)
    eye_d = nc.declare_dram_parameter("eye", [P, P], F32, isOutput=False)
    outT_d = nc.declare_dram_parameter("outT", [L, L], F32, isOutput=True)

    with tile.TileContext(nc) as tc:
        with (
            tc.tile_pool(name="drsc", bufs=1, space="DRAM") as drsc,
            tc.tile_pool(name="top", bufs=1) as top,
        ):
            dc_d = drsc.tile([DX, DX], F32, tag="dc")
            v_d = drsc.tile([L, DA], BF16, tag="vmat")

            xT = top.tile([P, NTX, L], F32R)
            eye = top.tile([P, P], F32)
            nc.sync.dma_start(eye[:], eye_d[:, :])
            ones_col = top.tile([P, 1], F32)
            nc.vector.memset(ones_col[:], 1.0)
            ones_row = top.tile([1, P], F32)
            nc.vector.memset(ones_row[:], 1.0)
            thous = top.tile([P, 1], F32)
            nc.vector.memset(thous[:], 1000.0)

            du_r = top.tile([P, NTX], F32R)   # f32r(u - c)
            wc_pc = top.tile([P, NTX], F32)   # (w - c) per-partition pieces
            c_all = top.tile([P, 1], F32)
            c1 = top.tile([1, 1], F32)
            S_pc = top.tile([P, NTL], F32)    # exact col sums, per-partition

            # ================= Phase C =================
            with (
                tc.tile_pool(name="cwk", bufs=1) as cwk,
                tc.tile_pool(name="cwq", bufs=2) as cwq,
                tc.tile_pool(name="csb", bufs=1) as csb,
                tc.tile_pool(name="cst", bufs=1) as cst,
                tc.tile_pool(name="cdt", bufs=2) as cdt,
                tc.tile_pool(name="crp", bufs=2) as crp,
                tc.tile_pool(name="cps", bufs=2, space="PSUM") as cps,
                tc.tile_pool(name="cpsS", bufs=2, space="PSUM") as cpsS,
            ):
                # DMA priority: wq(at=0), all wk row-blocks, then wq(at>=1)
                wkh = cwk.tile([P, NTA, DX], F32R, tag="wkh")
                wq0 = cwq.tile([P, NTA, P], F32R, tag="wqh")
                nc.sync.dma_start(
                    wq0[:], wqT_d[:, 0:P].rearrange("(t p) a -> p t a", p=P))
                for db in range(NTA):
                    nc.sync.dma_start(wkh[:, db, :],
                                      wkT_d[db * P:(db + 1) * P, :])

                # f32r ones and per-tile f32r copies of C feed the w-colmeans
                # matmul at 1 cycle/row instead of fp32's 4 (w only needs to
                # be consistent between dC and wc, not exact)
                ones_col_r = cst.tile([P, 1], F32R)
                nc.vector.memset(ones_col_r[:].bitcast(F32), 1.0)
                accw = cpsS.tile([P, DX], F32, tag="sacc")

                C_sb = csb.tile([P, NTX, DX], F32)
                for at in range(NTX):
                    if at == 0:
                        wqh = wq0
                    else:
                        wqh = cwq.tile([P, NTA, P], F32R, tag="wqh")
                        nc.sync.dma_start(
                            wqh[:],
                            wqT_d[:, at * P:(at + 1) * P].rearrange(
                                "(t p) a -> p t a", p=P))
                    acc = cps.tile([P, DX], F32, tag="cacc")
                    for db in range(NTA):
                        for ci in range(DX // CH):
                            cs = slice(ci * CH, (ci + 1) * CH)
                            nc.tensor.matmul(acc[:, cs], wqh[:, db, :],
                                             wkh[:, db, cs],
                                             start=(db == 0),
                                             stop=(db == NTA - 1))
                    nc.scalar.copy(C_sb[:, at, :], acc[:])
                    cr = crp.tile([P, DX], F32R, tag="cr")
                    nc.vector.tensor_copy(cr[:], acc[:])
                    for ci in range(DX // CH):
                        cs = slice(ci * CH, (ci + 1) * CH)
                        nc.tensor.matmul(accw[:1, cs], ones_col_r[:],
                                         cr[:, cs],
                                         start=(at == 0),
                                         stop=(at == NTX - 1))

                # u = rowmeans(C)  (DVE, fp32)
                u_sb = cst.tile([P, NTX], F32)
                for at in range(NTX):
                    nc.vector.reduce_sum(u_sb[:, at:at + 1], C_sb[:, at, :],
                                         axis=AX.X)
                nc.vector.tensor_scalar_mul(u_sb[:], u_sb[:], 1.0 / DX)

                # w = colmeans(R(C))  (accumulated f32r above)
                w_row = cst.tile([1, DX], F32)
                nc.vector.tensor_scalar_mul(w_row[:], accw[:1, :], 1.0 / DX)

                # c = mean(u); broadcast down partitions
                usum = cst.tile([P, 1], F32)
                nc.vector.reduce_sum(usum[:], u_sb[:], axis=AX.X)
                cacc = cpsS.tile([P, DX], F32, tag="sacc")
                nc.tensor.matmul(cacc[:1, :1], usum[:], ones_col[:],
                                 start=True, stop=True)
                nc.vector.tensor_scalar_mul(c1[:], cacc[:1, :1], 1.0 / DX)
                crep = cpsS.tile([P, DX], F32, tag="sacc")
                nc.tensor.matmul(crep[:, :1], ones_row[:], c1[:],
                                 start=True, stop=True)
                nc.vector.tensor_copy(c_all[:], crep[:, :1])

                # du_r = f32r(u - c)
                du_f = cst.tile([P, NTX], F32)
                nc.vector.tensor_scalar(du_f[:], u_sb[:], c_all[:], None,
                                        op0=ALU.subtract)
                nc.vector.tensor_copy(du_r[:], du_f[:])

                # wc_pc = (w - c) transposed to per-partition layout
                wcol = cst.tile([P, NTX], F32)
                for bt in range(NTX):
                    pcw = cpsS.tile([P, DX], F32, tag="sacc")
                    nc.tensor.matmul(
                        pcw[:, :1], w_row[:, bt * P:(bt + 1) * P],
                        ones_row[:, 0:1], start=True, stop=True)
                    nc.vector.tensor_copy(wcol[:, bt:bt + 1], pcw[:, :1])
                nc.vector.tensor_scalar(wc_pc[:], wcol[:], c_all[:], None,
                                        op0=ALU.subtract)

                # W_rep = w replicated down partitions (fp32 exact)
                accW = cpsS.tile([P, DX], F32, tag="sacc")
                for ci in range(DX // CH):
                    cs = slice(ci * CH, (ci + 1) * CH)
                    nc.tensor.matmul(accW[:, cs], ones_row[:], w_row[:, cs],
                                     start=True, stop=True)
                W_rep = cst.tile([P, DX], F32)
                nc.scalar.copy(W_rep[:], accW[:])

                # dC = C - du - w  -> spill fp32 to DRAM (PE rounds at read)
                for at in range(NTX):
                    dtmp = cdt.tile([P, DX], F32, tag="dtmp")
                    nc.vector.scalar_tensor_tensor(
                        dtmp[:], C_sb[:, at, :],
                        du_r[:, at:at + 1].bitcast(F32),
                        W_rep[:], op0=ALU.subtract, op1=ALU.subtract)
                    nc.sync.dma_start(dc_d[at * P:(at + 1) * P, :], dtmp[:])

            # xT load (contiguous row blocks); queued after C's DMAs
            for at in range(NTX):
                nc.sync.dma_start(xT[:, at, :], xT_d[at * P:(at + 1) * P, :])

            # ================= Phase V (v = x @ wv, bf16 spill) ==========
            with (
                tc.tile_pool(name="wvp", bufs=2) as wvp,
                tc.tile_pool(name="vop", bufs=2) as vop,
                tc.tile_pool(name="vps", bufs=2, space="PSUM") as vps,
            ):
                for h in range(2):
                    hsl = slice(h * (DA // 2), (h + 1) * (DA // 2))
                    wvh = wvp.tile([P, NTX, DA // 2], F32R, tag="wvh")
                    for at in range(NTX):
                        nc.sync.dma_start(
                            wvh[:, at, :],
                            wv_d[at * P:(at + 1) * P, hsl])
                    for jt in range(NTL):
                        jsl = slice(jt * P, (jt + 1) * P)
                        accv = vps.tile([P, DA // 2], F32, tag="vacc")
                        for at in range(NTX):
                            for ci in range((DA // 2) // CH):
                                cs = slice(ci * CH, (ci + 1) * CH)
                                nc.tensor.matmul(
                                    accv[:, cs], xT[:, at, jsl],
                                    wvh[:, at, cs],
                                    start=(at == 0), stop=(at == NTX - 1))
                        vout = vop.tile([P, DA // 2], BF16, tag="vout")
                        nc.vector.tensor_copy(vout[:], accv[:])
                        nc.sync.dma_start(v_d[jsl, hsl], vout[:])

            # ============ x-stats (exact S via DVE) + small matmuls ======
            with (
                tc.tile_pool(name="mid", bufs=1) as mid,
            ):
                yd = mid.tile([P, NTX, L], F32R)
                # Extra contraction tile pair feeding S[j]*a_row[i] through
                # the J matmul: rows (Sh,Sh,Sl) x (ah,al,ah), rest zero.
                extS = mid.tile([P, L], F32R)
                extY = mid.tile([P, L], F32R)
                nc.vector.memset(extS[:].bitcast(F32), 0.0)
                nc.vector.memset(extY[:].bitcast(F32), 0.0)

                with (
                    tc.tile_pool(name="xst", bufs=2) as xst,
                    tc.tile_pool(name="sst", bufs=1) as sst,
                    tc.tile_pool(name="ydc", bufs=2) as ydc,
                ):
                  S_rep = sst.tile([P, L], F32)
                  with tc.tile_pool(name="sps", bufs=2, space="PSUM") as sps:
                    for it in range(NTL):
                        xn = xst.tile([P, DX], F32, tag="xn")
                        nc.sync.dma_start(xn[:],
                                          xN_d[it * P:(it + 1) * P, :])
                        nc.vector.reduce_sum(S_pc[:, it:it + 1], xn[:],
                                             axis=AX.X)
                    # S_row[0, it*128+j] = S_pc[j, it]  (exact fp32 transpose
                    # via identity matmul)
                    S_row = sst.tile([1, L], F32)
                    srow_ps = sps.tile([P, L], F32, tag="spsa")
                    for it in range(NTL):
                        nc.tensor.matmul(
                            srow_ps[:1, it * P:(it + 1) * P],
                            S_pc[:, it:it + 1], eye[:],
                            start=True, stop=True)
                    nc.vector.tensor_copy(S_row[:], srow_ps[:1, :])
                    # S_rep (exact fp32 replication)
                    reps = sps.tile([P, L], F32, tag="spsa")
                    for ci in range(L // CH):
                        cs = slice(ci * CH, (ci + 1) * CH)
                        nc.tensor.matmul(reps[:, cs], ones_row[:],
                                         S_row[:, cs], start=True, stop=True)
                    nc.scalar.copy(S_rep[:], reps[:])
                    # xdu = x @ du (f32r 1-pass)
                    accx = sps.tile([P, L], F32, tag="spsa")
                    for at in range(NTX):
                        for ci in range(L // CH):
                            cs = slice(ci * CH, (ci + 1) * CH)
                            nc.tensor.matmul(
                                accx[:1, cs], du_r[:, at:at + 1],
                                xT[:, at, cs],
                                start=(at == 0), stop=(at == NTX - 1))
                    a_row = sst.tile([1, L], F32)
                    nc.vector.scalar_tensor_tensor(
                        a_row[:], S_row[:], c1[:], accx[:1, :],
                        op0=ALU.mult, op1=ALU.add)

                    # Dekker-split S_row and a_row into 11-bit-clean high +
                    # exact low halves (PE f32r re-round is then a no-op),
                    # place into extS/extY rows: (Sh,Sh,Sl) x (ah,al,ah).
                    with tc.tile_pool(name="dkp", bufs=1) as dkp:
                        def dek(src, dst, r0, r1, r2):
                            # truncate mantissa to 11 explicit bits: h is
                            # exactly representable in f32r, lo = src - h
                            h = dkp.tile([1, L], F32, tag="dk_h")
                            nc.vector.tensor_scalar(
                                h[:].bitcast(I32), src[:].bitcast(I32),
                                -4096, None, op0=ALU.bitwise_and)
                            lo = dkp.tile([1, L], F32, tag="dk_l")
                            nc.vector.tensor_sub(lo[:], src[:], h[:])
                            nc.sync.dma_start(dst[r0:r0 + 1, :],
                                              h[:].bitcast(F32R))
                            nc.sync.dma_start(dst[r1:r1 + 1, :],
                                              h[:].bitcast(F32R))
                            nc.sync.dma_start(dst[r2:r2 + 1, :],
                                              lo[:].bitcast(F32R))

                        dek(S_row, extS, 0, 1, 2)
                        dek(a_row, extY, 0, 2, 1)

                  # ================= Phase Y =================
                  with tc.tile_pool(name="yps", bufs=2, space="PSUM") as yps:
                    for bt in range(NTX):
                        dch = ydc.tile([P, NTX, P], F32R, tag="dch")
                        nc.sync.dma_start(
                            dch[:],
                            dc_d[:, bt * P:(bt + 1) * P].rearrange(
                                "(t p) b -> p t b", p=P).bitcast(F32R))
                        acc = yps.tile([P, L], F32, tag="yacc")
                        for at in range(NTX):
                            for ci in range(L // CH):
                                cs = slice(ci * CH, (ci + 1) * CH)
                                nc.tensor.matmul(
                                    acc[:, cs], dch[:, at, :], xT[:, at, cs],
                                    start=(at == 0), stop=(at == NTX - 1))
                        # fold (w-c) x S_row; store f32r
                        nc.vector.scalar_tensor_tensor(
                            yd[:, bt, :], S_rep[:], wc_pc[:, bt:bt + 1],
                            acc[:], op0=ALU.mult, op1=ALU.add)

                # ================= Phase J =================
                with (
                    tc.tile_pool(name="jm", bufs=2) as jm,
                    tc.tile_pool(name="jv", bufs=2) as jv,
                    tc.tile_pool(name="jz", bufs=1) as jz,
                    tc.tile_pool(name="jo", bufs=1) as jo,
                    tc.tile_pool(name="js", bufs=2) as js,
                    tc.tile_pool(name="jps", bufs=2, space="PSUM") as jps,
                ):
                    for jt in range(NTL):
                        jsl = slice(jt * P, (jt + 1) * P)
                        mstrip = jm.tile([P, L], U8, tag="mstrip")
                        nc.sync.dma_start(mstrip[:], maskT_d[jsl, :])
                        vj = jv.tile([P, DA], BF16, tag="vj")
                        nc.sync.dma_start(vj[:], v_d[jsl, :])

                        acc_s = jps.tile([P, L], F32, tag="sacc")
                        for ci in range(L // CH):
                            cs = slice(ci * CH, (ci + 1) * CH)
                            nc.tensor.matmul(
                                acc_s[:, cs], extS[:, jsl], extY[:, cs],
                                start=True, stop=False)
                        for bt in range(NTX):
                            for ci in range(L // CH):
                                cs = slice(ci * CH, (ci + 1) * CH)
                                nc.tensor.matmul(
                                    acc_s[:, cs], xT[:, bt, jsl],
                                    yd[:, bt, cs],
                                    start=False, stop=(bt == NTX - 1))

                        # zm = (z + 1000) * mask  (DVE, reads PSUM; the -1000
                        # shift cancels exactly in exp(scale*(zm - max)))
                        zm = jz.tile([P, L], F32, tag="zm")
                        nc.vector.scalar_tensor_tensor(
                            zm[:], acc_s[:], thous[:], mstrip[:],
                            op0=ALU.add, op1=ALU.mult)
                        rmax = js.tile([P, 1], F32, tag="rmax")
                        nc.vector.reduce_max(rmax[:], zm[:], axis=AX.X)
                        bias = js.tile([P, 1], F32, tag="bias")
                        nc.vector.tensor_scalar_mul(bias[:], rmax[:], -SCALE)
                        sig = js.tile([P, 1], F32, tag="sig")
                        e = jz.tile([P, L], F32, tag="e")
                        nc.scalar.activation(e[:], zm[:], AF.Exp, bias=bias[:],
                                             scale=SCALE, accum_out=sig[:])
                        rinv = js.tile([P, 1], F32, tag="rinv")
                        nc.vector.reciprocal(rinv[:], sig[:])

                        outt = jo.tile([P, L], F32, tag="outt")
                        nc.vector.scalar_tensor_tensor(
                            outt[:], e[:], rinv[:], vj[:],
                            op0=ALU.mult, op1=ALU.mult)
                        nc.sync.dma_start(outT_d[jsl, :], outt[:])

    nc.compile()
    return nc


_NC = None


def _get_nc():
    global _NC
    if _NC is None:
        _NC = build()
    return _NC


def _make_in_maps(inputs):
    x = np.asarray(inputs["x"], dtype=np.float32)
    wq0 = np.asarray(inputs["wq"], dtype=np.float32)[0]
    wk0 = np.asarray(inputs["wk"], dtype=np.float32)[0]
    wv0 = np.ascontiguousarray(np.asarray(inputs["wv"], dtype=np.float32)[0])
    mask = np.asarray(inputs["mask"])
    wqT = np.ascontiguousarray(wq0.T)
    wkT = np.ascontiguousarray(wk0.T)
    eye = np.eye(P, dtype=np.float32)
    return [
        dict(
            xT=np.ascontiguousarray(x[b].T),
            xN=np.ascontiguousarray(x[b]),
            wqT=wqT, wkT=wkT, wv=wv0,
            maskT=np.ascontiguousarray(mask[b].T).astype(np.uint8),
            eye=eye,
        )
        for b in range(B)
    ]


def _gather(res):
    return np.stack(
        [res.results[b]["outT"].T for b in range(B)]).astype(np.float32)


def kernel(x, wq, wk, wv, mask):
    nc = _get_nc()
    in_maps = _make_in_maps(dict(x=x, wq=wq, wk=wk, wv=wv, mask=mask))
    res = run_bass_kernel_spmd(nc, in_maps, list(range(B)))
    return _gather(res)


if __name__ == "__main__":
    import tempfile
    from concourse.bass_utils import compile_bass_kernel
    nc = build()
    print("bass compile OK")
    with tempfile.TemporaryDirectory() as td:
        compile_bass_kernel(nc, td, "v6.neff")
    print("walrus compile OK")


# revision 14
# speedup vs baseline: 1.0307x; 1.0307x over previous
"""Trainium2 Bass kernel v6 for nn_MaskedSelfAttention (B=8, L=2048, DX=1024, DA=2048).

Data-parallel over B (one batch per core). s = x (wq wk^T) x^T via C-trick;
column softmax (done row-wise in transposed space); out = p * v^T.

v6 vs v5b: single-pass f32r everywhere (no h/l split pairs), mean-centering
(u,w,c) + exact column-sum (S) rank-1 reconstruction gives the precision
(emulated rel err 6.7e-3 vs gate 2e-2; v5b measured 5.8e-3):
    C  = R(wq) R(wk)^T          1-pass f32r      [DX,DX] fp32 in SBUF
    dC = C - (u-c) - w          spilled f32 to DRAM (PE rounds at read)
    v  = R(x) R(wv)             1-pass f32r, bf16 spill
    yd = dC^T R(x)^T + (w-c) x S_row   1-pass, stored f32r in SBUF
    sT = R(x) yd + S[j]*a_row[i]; a_row = c*S_row + x@du
    S_row EXACT from x natural layout via DVE free-axis reduce (this exactness
    cancels the coherent part of x's f32r rounding; everything else is sloppy)
    mask applied exactly via zm = (z1+1000)*m - 1000 (the -1000 folds into the
    exp bias, so masked entries get exactly -1000 as in the reference)
Phase order C -> V -> Y -> J keeps PE dense; x-stats stream during V.
"""
import sys
sys.path.insert(0, "/opt/trn_rl_repo")
import numpy as np

import concourse.bacc as bacc
import concourse.tile as tile
import concourse.mybir as mybir
from concourse.bass_utils import run_bass_kernel_spmd

dt = mybir.dt
F32 = dt.float32
F32R = dt.float32r
U8 = dt.uint8
BF16 = dt.bfloat16
I32 = dt.int32
AF = mybir.ActivationFunctionType
ALU = mybir.AluOpType
AX = mybir.AxisListType

B, L, DX, DA = 8, 2048, 1024, 2048
P = 128
NTL = L // P      # 16
NTX = DX // P     # 8
NTA = DA // P     # 16
CH = 512
SCALE = float(1.0 / np.sqrt(np.float32(DA)))


def build():
    nc = bacc.Bacc("TRN2", target_bir_lowering=False, debug=False, num_devices=8)

    xT_d = nc.declare_dram_parameter("xT", [DX, L], F32R, isOutput=False)
    xN_d = nc.declare_dram_parameter("xN", [L, DX], F32, isOutput=False)
    wqT_d = nc.declare_dram_parameter("wqT", [DA, DX], F32R, isOutput=False)
    wkT_d = nc.declare_dram_parameter("wkT", [DA, DX], F32R, isOutput=False)
    wv_d = nc.declare_dram_parameter("wv", [DX, DA], F32R, isOutput=False)
    maskT_d = nc.declare_dram_parameter("maskT", [L, L], U8, isOutput=False)
    eye_d = nc.declare_dram_parameter("eye", [P, P], F32, isOutput=False)
    outT_d = nc.declare_dram_parameter("outT", [L, L], F32, isOutput=True)

    with tile.TileContext(nc) as tc:
        with (
            tc.tile_pool(name="drsc", bufs=1, space="DRAM") as drsc,
            tc.tile_pool(name="top", bufs=1) as top,
        ):
            dc_d = drsc.tile([DX, DX], F32, tag="dc")
            v_d = drsc.tile([L, DA], BF16, tag="vmat")

            xT = top.tile([P, NTX, L], F32R)
            eye = top.tile([P, P], F32)
            nc.sync.dma_start(eye[:], eye_d[:, :])
            ones_col = top.tile([P, 1], F32)
            nc.vector.memset(ones_col[:], 1.0)
            ones_row = top.tile([1, P], F32)
            nc.vector.memset(ones_row[:], 1.0)
            thous = top.tile([P, 1], F32)
            nc.vector.memset(thous[:], 1000.0)

            du_r = top.tile([P, NTX], F32R)   # f32r(u - c)
            wc_pc = top.tile([P, NTX], F32)   # (w - c) per-partition pieces
            c_all = top.tile([P, 1], F32)
            c1 = top.tile([1, 1], F32)
            S_pc = top.tile([P, NTL], F32)    # exact col sums, per-partition

            # ================= Phase C =================
            with (
                tc.tile_pool(name="cwk", bufs=1) as cwk,
                tc.tile_pool(name="cwq", bufs=2) as cwq,
                tc.tile_pool(name="csb", bufs=1) as csb,
                tc.tile_pool(name="cst", bufs=1) as cst,
                tc.tile_pool(name="cdt", bufs=2) as cdt,
                tc.tile_pool(name="cps", bufs=2, space="PSUM") as cps,
                tc.tile_pool(name="cpsS", bufs=2, space="PSUM") as cpsS,
            ):
                # DMA priority: wq(at=0), all wk row-blocks, then wq(at>=1)
                wkh = cwk.tile([P, NTA, DX], F32R, tag="wkh")
                wq0 = cwq.tile([P, NTA, P], F32R, tag="wqh")
                nc.sync.dma_start(
                    wq0[:], wqT_d[:, 0:P].rearrange("(t p) a -> p t a", p=P))
                for db in range(NTA):
                    nc.sync.dma_start(wkh[:, db, :],
                                      wkT_d[db * P:(db + 1) * P, :])

                C_sb = csb.tile([P, NTX, DX], F32)
                for at in range(NTX):
                    if at == 0:
                        wqh = wq0
                    else:
                        wqh = cwq.tile([P, NTA, P], F32R, tag="wqh")
                        nc.sync.dma_start(
                            wqh[:],
                            wqT_d[:, at * P:(at + 1) * P].rearrange(
                                "(t p) a -> p t a", p=P))
                    acc = cps.tile([P, DX], F32, tag="cacc")
                    for db in range(NTA):
                        for ci in range(DX // CH):
                            cs = slice(ci * CH, (ci + 1) * CH)
                            nc.tensor.matmul(acc[:, cs], wqh[:, db, :],
                                             wkh[:, db, cs],
                                             start=(db == 0),
                                             stop=(db == NTA - 1))
                    nc.scalar.copy(C_sb[:, at, :], acc[:])

                # u = rowmeans(C)  (DVE, fp32)
                u_sb = cst.tile([P, NTX], F32)
                for at in range(NTX):
                    nc.vector.reduce_sum(u_sb[:, at:at + 1], C_sb[:, at, :],
                                         axis=AX.X)
                nc.vector.tensor_scalar_mul(u_sb[:], u_sb[:], 1.0 / DX)

                # w = colmeans(C)  (fp32 ones matmul)
                w_row = cst.tile([1, DX], F32)
                accw = cpsS.tile([P, DX], F32, tag="sacc")
                for at in range(NTX):
                    for ci in range(DX // CH):
                        cs = slice(ci * CH, (ci + 1) * CH)
                        nc.tensor.matmul(
                            accw[:1, cs], ones_col[:], C_sb[:, at, cs],
                            start=(at == 0), stop=(at == NTX - 1))
                nc.vector.tensor_scalar_mul(w_row[:], accw[:1, :], 1.0 / DX)

                # c = mean(u); broadcast down partitions
                usum = cst.tile([P, 1], F32)
                nc.vector.reduce_sum(usum[:], u_sb[:], axis=AX.X)
                cacc = cpsS.tile([P, DX], F32, tag="sacc")
                nc.tensor.matmul(cacc[:1, :1], usum[:], ones_col[:],
                                 start=True, stop=True)
                nc.vector.tensor_scalar_mul(c1[:], cacc[:1, :1], 1.0 / DX)
                crep = cpsS.tile([P, DX], F32, tag="sacc")
                nc.tensor.matmul(crep[:, :1], ones_row[:], c1[:],
                                 start=True, stop=True)
                nc.vector.tensor_copy(c_all[:], crep[:, :1])

                # du_r = f32r(u - c)
                du_f = cst.tile([P, NTX], F32)
                nc.vector.tensor_scalar(du_f[:], u_sb[:], c_all[:], None,
                                        op0=ALU.subtract)
                nc.vector.tensor_copy(du_r[:], du_f[:])

                # wc_pc = (w - c) transposed to per-partition layout
                wcol = cst.tile([P, NTX], F32)
                for bt in range(NTX):
                    pcw = cpsS.tile([P, DX], F32, tag="sacc")
                    nc.tensor.matmul(
                        pcw[:, :1], w_row[:, bt * P:(bt + 1) * P],
                        ones_row[:, 0:1], start=True, stop=True)
                    nc.vector.tensor_copy(wcol[:, bt:bt + 1], pcw[:, :1])
                nc.vector.tensor_scalar(wc_pc[:], wcol[:], c_all[:], None,
                                        op0=ALU.subtract)

                # W_rep = w replicated down partitions (fp32 exact)
                accW = cpsS.tile([P, DX], F32, tag="sacc")
                for ci in range(DX // CH):
                    cs = slice(ci * CH, (ci + 1) * CH)
                    nc.tensor.matmul(accW[:, cs], ones_row[:], w_row[:, cs],
                                     start=True, stop=True)
                W_rep = cst.tile([P, DX], F32)
                nc.scalar.copy(W_rep[:], accW[:])

                # dC = C - du - w  -> spill fp32 to DRAM (PE rounds at read)
                for at in range(NTX):
                    dtmp = cdt.tile([P, DX], F32, tag="dtmp")
                    nc.vector.scalar_tensor_tensor(
                        dtmp[:], C_sb[:, at, :],
                        du_r[:, at:at + 1].bitcast(F32),
                        W_rep[:], op0=ALU.subtract, op1=ALU.subtract)
                    nc.sync.dma_start(dc_d[at * P:(at + 1) * P, :], dtmp[:])

            # xT load (contiguous row blocks); queued after C's DMAs
            for at in range(NTX):
                nc.sync.dma_start(xT[:, at, :], xT_d[at * P:(at + 1) * P, :])

            # ================= Phase V (v = x @ wv, bf16 spill) ==========
            with (
                tc.tile_pool(name="wvp", bufs=2) as wvp,
                tc.tile_pool(name="vop", bufs=2) as vop,
                tc.tile_pool(name="vps", bufs=2, space="PSUM") as vps,
            ):
                for h in range(2):
                    hsl = slice(h * (DA // 2), (h + 1) * (DA // 2))
                    wvh = wvp.tile([P, NTX, DA // 2], F32R, tag="wvh")
                    for at in range(NTX):
                        nc.sync.dma_start(
                            wvh[:, at, :],
                            wv_d[at * P:(at + 1) * P, hsl])
                    for jt in range(NTL):
                        jsl = slice(jt * P, (jt + 1) * P)
                        accv = vps.tile([P, DA // 2], F32, tag="vacc")
                        for at in range(NTX):
                            for ci in range((DA // 2) // CH):
                                cs = slice(ci * CH, (ci + 1) * CH)
                                nc.tensor.matmul(
                                    accv[:, cs], xT[:, at, jsl],
                                    wvh[:, at, cs],
                                    start=(at == 0), stop=(at == NTX - 1))
                        vout = vop.tile([P, DA // 2], BF16, tag="vout")
                        nc.vector.tensor_copy(vout[:], accv[:])
                        nc.sync.dma_start(v_d[jsl, hsl], vout[:])

            # ============ x-stats (exact S via DVE) + small matmuls ======
            with (
                tc.tile_pool(name="mid", bufs=1) as mid,
            ):
                yd = mid.tile([P, NTX, L], F32R)
                # Extra contraction tile pair feeding S[j]*a_row[i] through
                # the J matmul: rows (Sh,Sh,Sl) x (ah,al,ah), rest zero.
                extS = mid.tile([P, L], F32R)
                extY = mid.tile([P, L], F32R)
                nc.vector.memset(extS[:].bitcast(F32), 0.0)
                nc.vector.memset(extY[:].bitcast(F32), 0.0)

                with (
                    tc.tile_pool(name="xst", bufs=2) as xst,
                    tc.tile_pool(name="sst", bufs=1) as sst,
                    tc.tile_pool(name="ydc", bufs=2) as ydc,
                ):
                  S_rep = sst.tile([P, L], F32)
                  with tc.tile_pool(name="sps", bufs=2, space="PSUM") as sps:
                    for it in range(NTL):
                        xn = xst.tile([P, DX], F32, tag="xn")
                        nc.sync.dma_start(xn[:],
                                          xN_d[it * P:(it + 1) * P, :])
                        nc.vector.reduce_sum(S_pc[:, it:it + 1], xn[:],
                                             axis=AX.X)
                    # S_row[0, it*128+j] = S_pc[j, it]  (exact fp32 transpose
                    # via identity matmul)
                    S_row = sst.tile([1, L], F32)
                    srow_ps = sps.tile([P, L], F32, tag="spsa")
                    for it in range(NTL):
                        nc.tensor.matmul(
                            srow_ps[:1, it * P:(it + 1) * P],
                            S_pc[:, it:it + 1], eye[:],
                            start=True, stop=True)
                    nc.vector.tensor_copy(S_row[:], srow_ps[:1, :])
                    # S_rep (exact fp32 replication)
                    reps = sps.tile([P, L], F32, tag="spsa")
                    for ci in range(L // CH):
                        cs = slice(ci * CH, (ci + 1) * CH)
                        nc.tensor.matmul(reps[:, cs], ones_row[:],
                                         S_row[:, cs], start=True, stop=True)
                    nc.scalar.copy(S_rep[:], reps[:])
                    # xdu = x @ du (f32r 1-pass)
                    accx = sps.tile([P, L], F32, tag="spsa")
                    for at in range(NTX):
                        for ci in range(L // CH):
                            cs = slice(ci * CH, (ci + 1) * CH)
                            nc.tensor.matmul(
                                accx[:1, cs], du_r[:, at:at + 1],
                                xT[:, at, cs],
                                start=(at == 0), stop=(at == NTX - 1))
                    a_row = sst.tile([1, L], F32)
                    nc.vector.scalar_tensor_tensor(
                        a_row[:], S_row[:], c1[:], accx[:1, :],
                        op0=ALU.mult, op1=ALU.add)

                    # Dekker-split S_row and a_row into 11-bit-clean high +
                    # exact low halves (PE f32r re-round is then a no-op),
                    # place into extS/extY rows: (Sh,Sh,Sl) x (ah,al,ah).
                    with tc.tile_pool(name="dkp", bufs=1) as dkp:
                        def dek(src, dst, r0, r1, r2):
                            # truncate mantissa to 11 explicit bits: h is
                            # exactly representable in f32r, lo = src - h
                            h = dkp.tile([1, L], F32, tag="dk_h")
                            nc.vector.tensor_scalar(
                                h[:].bitcast(I32), src[:].bitcast(I32),
                                -4096, None, op0=ALU.bitwise_and)
                            lo = dkp.tile([1, L], F32, tag="dk_l")
                            nc.vector.tensor_sub(lo[:], src[:], h[:])
                            nc.sync.dma_start(dst[r0:r0 + 1, :],
                                              h[:].bitcast(F32R))
                            nc.sync.dma_start(dst[r1:r1 + 1, :],
                                              h[:].bitcast(F32R))
                            nc.sync.dma_start(dst[r2:r2 + 1, :],
                                              lo[:].bitcast(F32R))

                        dek(S_row, extS, 0, 1, 2)
                        dek(a_row, extY, 0, 2, 1)

                  # ================= Phase Y =================
                  with tc.tile_pool(name="yps", bufs=2, space="PSUM") as yps:
                    for bt in range(NTX):
                        dch = ydc.tile([P, NTX, P], F32R, tag="dch")
                        nc.sync.dma_start(
                            dch[:],
                            dc_d[:, bt * P:(bt + 1) * P].rearrange(
                                "(t p) b -> p t b", p=P).bitcast(F32R))
                        acc = yps.tile([P, L], F32, tag="yacc")
                        for at in range(NTX):
                            for ci in range(L // CH):
                                cs = slice(ci * CH, (ci + 1) * CH)
                                nc.tensor.matmul(
                                    acc[:, cs], dch[:, at, :], xT[:, at, cs],
                                    start=(at == 0), stop=(at == NTX - 1))
                        # fold (w-c) x S_row; store f32r
                        nc.vector.scalar_tensor_tensor(
                            yd[:, bt, :], S_rep[:], wc_pc[:, bt:bt + 1],
                            acc[:], op0=ALU.mult, op1=ALU.add)

                # ================= Phase J =================
                with (
                    tc.tile_pool(name="jm", bufs=2) as jm,
                    tc.tile_pool(name="jv", bufs=2) as jv,
                    tc.tile_pool(name="jz", bufs=1) as jz,
                    tc.tile_pool(name="jo", bufs=1) as jo,
                    tc.tile_pool(name="js", bufs=2) as js,
                    tc.tile_pool(name="jps", bufs=2, space="PSUM") as jps,
                ):
                    for jt in range(NTL):
                        jsl = slice(jt * P, (jt + 1) * P)
                        mstrip = jm.tile([P, L], U8, tag="mstrip")
                        nc.sync.dma_start(mstrip[:], maskT_d[jsl, :])
                        vj = jv.tile([P, DA], BF16, tag="vj")
                        nc.sync.dma_start(vj[:], v_d[jsl, :])

                        acc_s = jps.tile([P, L], F32, tag="sacc")
                        for ci in range(L // CH):
                            cs = slice(ci * CH, (ci + 1) * CH)
                            nc.tensor.matmul(
                                acc_s[:, cs], extS[:, jsl], extY[:, cs],
                                start=True, stop=False)
                        for bt in range(NTX):
                            for ci in range(L // CH):
                                cs = slice(ci * CH, (ci + 1) * CH)
                                nc.tensor.matmul(
                                    acc_s[:, cs], xT[:, bt, jsl],
                                    yd[:, bt, cs],
                                    start=False, stop=(bt == NTX - 1))

                        # zm = (z + 1000) * mask  (DVE, reads PSUM; the -1000
                        # shift cancels exactly in exp(scale*(zm - max)))
                        zm = jz.tile([P, L], F32, tag="zm")
                        nc.vector.scalar_tensor_tensor(
                            zm[:], acc_s[:], thous[:], mstrip[:],
                            op0=ALU.add, op1=ALU.mult)
                        rmax = js.tile([P, 1], F32, tag="rmax")
                        nc.vector.reduce_max(rmax[:], zm[:], axis=AX.X)
                        bias = js.tile([P, 1], F32, tag="bias")
                        nc.vector.tensor_scalar_mul(bias[:], rmax[:], -SCALE)
                        sig = js.tile([P, 1], F32, tag="sig")
                        e = jz.tile([P, L], F32, tag="e")
                        nc.scalar.activation(e[:], zm[:], AF.Exp, bias=bias[:],
                                             scale=SCALE, accum_out=sig[:])
                        rinv = js.tile([P, 1], F32, tag="rinv")
                        nc.vector.reciprocal(rinv[:], sig[:])

                        outt = jo.tile([P, L], F32, tag="outt")
                        nc.vector.scalar_tensor_tensor(
                            outt[:], e[:], rinv[:], vj[:],
                            op0=ALU.mult, op1=ALU.mult)
                        nc.sync.dma_start(outT_d[jsl, :], outt[:])

    nc.compile()
    return nc


_NC = None


def _get_nc():
    global _NC
    if _NC is None:
        _NC = build()
    return _NC


def _make_in_maps(inputs):
    x = np.asarray(inputs["x"], dtype=np.float32)
    wq0 = np.asarray(inputs["wq"], dtype=np.float32)[0]
    wk0 = np.asarray(inputs["wk"], dtype=np.float32)[0]
    wv0 = np.ascontiguousarray(np.asarray(inputs["wv"], dtype=np.float32)[0])
    mask = np.asarray(inputs["mask"])
    wqT = np.ascontiguousarray(wq0.T)
    wkT = np.ascontiguousarray(wk0.T)
    eye = np.eye(P, dtype=np.float32)
    return [
        dict(
            xT=np.ascontiguousarray(x[b].T),
            xN=np.ascontiguousarray(x[b]),
            wqT=wqT, wkT=wkT, wv=wv0,
            maskT=np.ascontiguousarray(mask[b].T).astype(np.uint8),
            eye=eye,
        )
        for b in range(B)
    ]


def _gather(res):
    return np.stack(
        [res.results[b]["outT"].T for b in range(B)]).astype(np.float32)


def kernel(x, wq, wk, wv, mask):
    nc = _get_nc()
    in_maps = _make_in_maps(dict(x=x, wq=wq, wk=wk, wv=wv, mask=mask))
    res = run_bass_kernel_spmd(nc, in_maps, list(range(B)))
    return _gather(res)


if __name__ == "__main__":
    import tempfile
    from concourse.bass_utils import compile_bass_kernel
    nc = build()
    print("bass compile OK")
    with tempfile.TemporaryDirectory() as td:
        compile_bass_kernel(nc, td, "v6.neff")
    print("walrus compile OK")


# revision 15
# speedup vs baseline: 1.0354x; 1.0046x over previous
"""Trainium2 Bass kernel v6 for nn_MaskedSelfAttention (B=8, L=2048, DX=1024, DA=2048).

Data-parallel over B (one batch per core). s = x (wq wk^T) x^T via C-trick;
column softmax (done row-wise in transposed space); out = p * v^T.

v6 vs v5b: single-pass f32r everywhere (no h/l split pairs), mean-centering
(u,w,c) + exact column-sum (S) rank-1 reconstruction gives the precision
(emulated rel err 6.7e-3 vs gate 2e-2; v5b measured 5.8e-3):
    C  = R(wq) R(wk)^T          1-pass f32r      [DX,DX] fp32 in SBUF
    dC = C - (u-c) - w          spilled f32 to DRAM (PE rounds at read)
    v  = R(x) R(wv)             1-pass f32r, bf16 spill
    yd = dC^T R(x)^T + (w-c) x S_row   1-pass, stored f32r in SBUF
    sT = R(x) yd + S[j]*a_row[i]; a_row = c*S_row + x@du
    S_row EXACT from x natural layout via DVE free-axis reduce (this exactness
    cancels the coherent part of x's f32r rounding; everything else is sloppy)
    mask applied exactly via zm = (z1+1000)*m - 1000 (the -1000 folds into the
    exp bias, so masked entries get exactly -1000 as in the reference)
Phase order C -> V -> Y -> J keeps PE dense; x-stats stream during V.
"""
import sys
sys.path.insert(0, "/opt/trn_rl_repo")
import numpy as np

import concourse.bacc as bacc
import concourse.tile as tile
import concourse.mybir as mybir
from concourse.bass_utils import run_bass_kernel_spmd

dt = mybir.dt
F32 = dt.float32
F32R = dt.float32r
U8 = dt.uint8
BF16 = dt.bfloat16
I32 = dt.int32
AF = mybir.ActivationFunctionType
ALU = mybir.AluOpType
AX = mybir.AxisListType

B, L, DX, DA = 8, 2048, 1024, 2048
P = 128
NTL = L // P      # 16
NTX = DX // P     # 8
NTA = DA // P     # 16
CH = 512
SCALE = float(1.0 / np.sqrt(np.float32(DA)))


def build():
    nc = bacc.Bacc("TRN2", target_bir_lowering=False, debug=False, num_devices=8)

    xT_d = nc.declare_dram_parameter("xT", [DX, L], F32R, isOutput=False)
    xN_d = nc.declare_dram_parameter("xN", [L, DX], F32, isOutput=False)
    wqT_d = nc.declare_dram_parameter("wqT", [DA, DX], F32R, isOutput=False)
    wkT_d = nc.declare_dram_parameter("wkT", [DA, DX], F32R, isOutput=False)
    wv_d = nc.declare_dram_parameter("wv", [DX, DA], F32R, isOutput=False)
    maskT_d = nc.declare_dram_parameter("maskT", [L, L], U8, isOutput=False)
    eye_d = nc.declare_dram_parameter("eye", [P, P], F32, isOutput=False)
    outT_d = nc.declare_dram_parameter("outT", [L, L], F32, isOutput=True)

    with tile.TileContext(nc) as tc:
        with (
            tc.tile_pool(name="drsc", bufs=1, space="DRAM") as drsc,
            tc.tile_pool(name="top", bufs=1) as top,
        ):
            dc_d = drsc.tile([DX, DX], F32, tag="dc")
            v_d = drsc.tile([L, DA], BF16, tag="vmat")

            xT = top.tile([P, NTX, L], F32R)
            eye = top.tile([P, P], F32)
            nc.sync.dma_start(eye[:], eye_d[:, :])
            ones_col = top.tile([P, 1], F32)
            nc.vector.memset(ones_col[:], 1.0)
            ones_row = top.tile([1, P], F32)
            nc.vector.memset(ones_row[:], 1.0)
            thous = top.tile([P, 1], F32)
            nc.vector.memset(thous[:], 1000.0)

            du_r = top.tile([P, NTX], F32R)   # f32r(u - c)
            wc_pc = top.tile([P, NTX], F32)   # (w - c) per-partition pieces
            c_all = top.tile([P, 1], F32)
            c1 = top.tile([1, 1], F32)
            S_pc = top.tile([P, NTL], F32)    # exact col sums, per-partition

            # ================= Phase C =================
            with (
                tc.tile_pool(name="cwk", bufs=1) as cwk,
                tc.tile_pool(name="cwq", bufs=2) as cwq,
                tc.tile_pool(name="csb", bufs=1) as csb,
                tc.tile_pool(name="cst", bufs=1) as cst,
                tc.tile_pool(name="cdt", bufs=2) as cdt,
                tc.tile_pool(name="cps", bufs=2, space="PSUM") as cps,
                tc.tile_pool(name="cpsS", bufs=2, space="PSUM") as cpsS,
            ):
                # DMA priority: wq(at=0), all wk row-blocks, then wq(at>=1)
                wkh = cwk.tile([P, NTA, DX], F32R, tag="wkh")
                wq0 = cwq.tile([P, NTA, P], F32R, tag="wqh")
                nc.sync.dma_start(
                    wq0[:], wqT_d[:, 0:P].rearrange("(t p) a -> p t a", p=P))
                for db in range(NTA):
                    nc.sync.dma_start(wkh[:, db, :],
                                      wkT_d[db * P:(db + 1) * P, :])

                C_sb = csb.tile([P, NTX, DX], F32)
                for at in range(NTX):
                    if at == 0:
                        wqh = wq0
                    else:
                        wqh = cwq.tile([P, NTA, P], F32R, tag="wqh")
                        nc.sync.dma_start(
                            wqh[:],
                            wqT_d[:, at * P:(at + 1) * P].rearrange(
                                "(t p) a -> p t a", p=P))
                    acc = cps.tile([P, DX], F32, tag="cacc")
                    for db in range(NTA):
                        for ci in range(DX // CH):
                            cs = slice(ci * CH, (ci + 1) * CH)
                            nc.tensor.matmul(acc[:, cs], wqh[:, db, :],
                                             wkh[:, db, cs],
                                             start=(db == 0),
                                             stop=(db == NTA - 1))
                    nc.scalar.copy(C_sb[:, at, :], acc[:])

                # u = rowmeans(C)  (DVE, fp32)
                u_sb = cst.tile([P, NTX], F32)
                for at in range(NTX):
                    nc.vector.reduce_sum(u_sb[:, at:at + 1], C_sb[:, at, :],
                                         axis=AX.X)
                nc.vector.tensor_scalar_mul(u_sb[:], u_sb[:], 1.0 / DX)

                # w = colmeans(C)  (fp32 ones matmul)
                w_row = cst.tile([1, DX], F32)
                accw = cpsS.tile([P, DX], F32, tag="sacc")
                for at in range(NTX):
                    for ci in range(DX // CH):
                        cs = slice(ci * CH, (ci + 1) * CH)
                        nc.tensor.matmul(
                            accw[:1, cs], ones_col[:], C_sb[:, at, cs],
                            start=(at == 0), stop=(at == NTX - 1))
                nc.vector.tensor_scalar_mul(w_row[:], accw[:1, :], 1.0 / DX)

                # c = mean(u); broadcast down partitions
                usum = cst.tile([P, 1], F32)
                nc.vector.reduce_sum(usum[:], u_sb[:], axis=AX.X)
                cacc = cpsS.tile([P, DX], F32, tag="sacc")
                nc.tensor.matmul(cacc[:1, :1], usum[:], ones_col[:],
                                 start=True, stop=True)
                nc.vector.tensor_scalar_mul(c1[:], cacc[:1, :1], 1.0 / DX)
                crep = cpsS.tile([P, DX], F32, tag="sacc")
                nc.tensor.matmul(crep[:, :1], ones_row[:], c1[:],
                                 start=True, stop=True)
                nc.vector.tensor_copy(c_all[:], crep[:, :1])

                # du_r = f32r(u - c)
                du_f = cst.tile([P, NTX], F32)
                nc.vector.tensor_scalar(du_f[:], u_sb[:], c_all[:], None,
                                        op0=ALU.subtract)
                nc.vector.tensor_copy(du_r[:], du_f[:])

                # wc_pc = (w - c) transposed to per-partition layout
                wcol = cst.tile([P, NTX], F32)
                for bt in range(NTX):
                    pcw = cpsS.tile([P, DX], F32, tag="sacc")
                    nc.tensor.matmul(
                        pcw[:, :1], w_row[:, bt * P:(bt + 1) * P],
                        ones_row[:, 0:1], start=True, stop=True)
                    nc.vector.tensor_copy(wcol[:, bt:bt + 1], pcw[:, :1])
                nc.vector.tensor_scalar(wc_pc[:], wcol[:], c_all[:], None,
                                        op0=ALU.subtract)

                # W_rep = w replicated down partitions (fp32 exact)
                accW = cpsS.tile([P, DX], F32, tag="sacc")
                for ci in range(DX // CH):
                    cs = slice(ci * CH, (ci + 1) * CH)
                    nc.tensor.matmul(accW[:, cs], ones_row[:], w_row[:, cs],
                                     start=True, stop=True)
                W_rep = cst.tile([P, DX], F32)
                nc.scalar.copy(W_rep[:], accW[:])

                # dC = C - du - w  -> spill fp32 to DRAM (PE rounds at read)
                for at in range(NTX):
                    dtmp = cdt.tile([P, DX], F32, tag="dtmp")
                    nc.vector.scalar_tensor_tensor(
                        dtmp[:], C_sb[:, at, :],
                        du_r[:, at:at + 1].bitcast(F32),
                        W_rep[:], op0=ALU.subtract, op1=ALU.subtract)
                    nc.sync.dma_start(dc_d[at * P:(at + 1) * P, :], dtmp[:])

            # xT load (contiguous row blocks); queued after C's DMAs
            for at in range(NTX):
                nc.sync.dma_start(xT[:, at, :], xT_d[at * P:(at + 1) * P, :])

            # ================= Phase V (v = x @ wv, bf16 spill) ==========
            with (
                tc.tile_pool(name="wvp", bufs=2) as wvp,
                tc.tile_pool(name="vop", bufs=2) as vop,
                tc.tile_pool(name="vps", bufs=2, space="PSUM") as vps,
            ):
                for h in range(2):
                    hsl = slice(h * (DA // 2), (h + 1) * (DA // 2))
                    wvh = wvp.tile([P, NTX, DA // 2], F32R, tag="wvh")
                    for at in range(NTX):
                        nc.sync.dma_start(
                            wvh[:, at, :],
                            wv_d[at * P:(at + 1) * P, hsl])
                    for jt in range(NTL):
                        jsl = slice(jt * P, (jt + 1) * P)
                        accv = vps.tile([P, DA // 2], F32, tag="vacc")
                        for at in range(NTX):
                            for ci in range((DA // 2) // CH):
                                cs = slice(ci * CH, (ci + 1) * CH)
                                nc.tensor.matmul(
                                    accv[:, cs], xT[:, at, jsl],
                                    wvh[:, at, cs],
                                    start=(at == 0), stop=(at == NTX - 1))
                        vout = vop.tile([P, DA // 2], BF16, tag="vout")
                        nc.vector.tensor_copy(vout[:], accv[:])
                        nc.sync.dma_start(v_d[jsl, hsl], vout[:])

            # ============ x-stats (exact S via DVE) + small matmuls ======
            with (
                tc.tile_pool(name="mid", bufs=1) as mid,
            ):
                yd = mid.tile([P, NTX, L], F32R)
                # Extra contraction tile pair feeding S[j]*a_row[i] through
                # the J matmul: rows (Sh,Sh,Sl) x (ah,al,ah), rest zero.
                extS = mid.tile([P, L], F32R)
                extY = mid.tile([P, L], F32R)
                nc.vector.memset(extS[:].bitcast(F32), 0.0)
                nc.vector.memset(extY[:].bitcast(F32), 0.0)

                with (
                    tc.tile_pool(name="xst", bufs=2) as xst,
                    tc.tile_pool(name="sst", bufs=1) as sst,
                    tc.tile_pool(name="ydc", bufs=2) as ydc,
                ):
                  S_rep = sst.tile([P, L], F32)
                  with tc.tile_pool(name="sps", bufs=2, space="PSUM") as sps:
                    xjunk = sst.tile([P, DX], F32)
                    for it in range(NTL):
                        xn = xst.tile([P, DX], F32, tag="xn")
                        nc.sync.dma_start(xn[:],
                                          xN_d[it * P:(it + 1) * P, :])
                        # free-axis sum on the (idle) ACT accumulator so the
                        # reduces run during V instead of queueing on DVE
                        nc.scalar.activation(xjunk[:], xn[:], AF.Copy,
                                             accum_out=S_pc[:, it:it + 1])
                    # S_row[0, it*128+j] = S_pc[j, it]  (exact fp32 transpose
                    # via identity matmul)
                    S_row = sst.tile([1, L], F32)
                    srow_ps = sps.tile([P, L], F32, tag="spsa")
                    for it in range(NTL):
                        nc.tensor.matmul(
                            srow_ps[:1, it * P:(it + 1) * P],
                            S_pc[:, it:it + 1], eye[:],
                            start=True, stop=True)
                    nc.vector.tensor_copy(S_row[:], srow_ps[:1, :])
                    # S_rep (exact fp32 replication)
                    reps = sps.tile([P, L], F32, tag="spsa")
                    for ci in range(L // CH):
                        cs = slice(ci * CH, (ci + 1) * CH)
                        nc.tensor.matmul(reps[:, cs], ones_row[:],
                                         S_row[:, cs], start=True, stop=True)
                    nc.scalar.copy(S_rep[:], reps[:])
                    # xdu = x @ du (f32r 1-pass)
                    accx = sps.tile([P, L], F32, tag="spsa")
                    for at in range(NTX):
                        for ci in range(L // CH):
                            cs = slice(ci * CH, (ci + 1) * CH)
                            nc.tensor.matmul(
                                accx[:1, cs], du_r[:, at:at + 1],
                                xT[:, at, cs],
                                start=(at == 0), stop=(at == NTX - 1))
                    a_row = sst.tile([1, L], F32)
                    nc.vector.scalar_tensor_tensor(
                        a_row[:], S_row[:], c1[:], accx[:1, :],
                        op0=ALU.mult, op1=ALU.add)

                    # Dekker-split S_row and a_row into 11-bit-clean high +
                    # exact low halves (PE f32r re-round is then a no-op),
                    # place into extS/extY rows: (Sh,Sh,Sl) x (ah,al,ah).
                    with tc.tile_pool(name="dkp", bufs=1) as dkp:
                        def dek(src, dst, r0, r1, r2):
                            # truncate mantissa to 11 explicit bits: h is
                            # exactly representable in f32r, lo = src - h
                            h = dkp.tile([1, L], F32, tag="dk_h")
                            nc.vector.tensor_scalar(
                                h[:].bitcast(I32), src[:].bitcast(I32),
                                -4096, None, op0=ALU.bitwise_and)
                            lo = dkp.tile([1, L], F32, tag="dk_l")
                            nc.vector.tensor_sub(lo[:], src[:], h[:])
                            nc.sync.dma_start(dst[r0:r0 + 1, :],
                                              h[:].bitcast(F32R))
                            nc.sync.dma_start(dst[r1:r1 + 1, :],
                                              h[:].bitcast(F32R))
                            nc.sync.dma_start(dst[r2:r2 + 1, :],
                                              lo[:].bitcast(F32R))

                        dek(S_row, extS, 0, 1, 2)
                        dek(a_row, extY, 0, 2, 1)

                  # ================= Phase Y =================
                  with tc.tile_pool(name="yps", bufs=2, space="PSUM") as yps:
                    for bt in range(NTX):
                        dch = ydc.tile([P, NTX, P], F32R, tag="dch")
                        nc.sync.dma_start(
                            dch[:],
                            dc_d[:, bt * P:(bt + 1) * P].rearrange(
                                "(t p) b -> p t b", p=P).bitcast(F32R))
                        acc = yps.tile([P, L], F32, tag="yacc")
                        for at in range(NTX):
                            for ci in range(L // CH):
                                cs = slice(ci * CH, (ci + 1) * CH)
                                nc.tensor.matmul(
                                    acc[:, cs], dch[:, at, :], xT[:, at, cs],
                                    start=(at == 0), stop=(at == NTX - 1))
                        # fold (w-c) x S_row; store f32r
                        nc.vector.scalar_tensor_tensor(
                            yd[:, bt, :], S_rep[:], wc_pc[:, bt:bt + 1],
                            acc[:], op0=ALU.mult, op1=ALU.add)

                # ================= Phase J =================
                with (
                    tc.tile_pool(name="jm", bufs=2) as jm,
                    tc.tile_pool(name="jv", bufs=2) as jv,
                    tc.tile_pool(name="jz", bufs=1) as jz,
                    tc.tile_pool(name="jo", bufs=1) as jo,
                    tc.tile_pool(name="js", bufs=2) as js,
                    tc.tile_pool(name="jps", bufs=2, space="PSUM") as jps,
                ):
                    for jt in range(NTL):
                        jsl = slice(jt * P, (jt + 1) * P)
                        mstrip = jm.tile([P, L], U8, tag="mstrip")
                        nc.sync.dma_start(mstrip[:], maskT_d[jsl, :])
                        vj = jv.tile([P, DA], BF16, tag="vj")
                        nc.sync.dma_start(vj[:], v_d[jsl, :])

                        acc_s = jps.tile([P, L], F32, tag="sacc")
                        for ci in range(L // CH):
                            cs = slice(ci * CH, (ci + 1) * CH)
                            nc.tensor.matmul(
                                acc_s[:, cs], extS[:, jsl], extY[:, cs],
                                start=True, stop=False)
                        for bt in range(NTX):
                            for ci in range(L // CH):
                                cs = slice(ci * CH, (ci + 1) * CH)
                                nc.tensor.matmul(
                                    acc_s[:, cs], xT[:, bt, jsl],
                                    yd[:, bt, cs],
                                    start=False, stop=(bt == NTX - 1))

                        # zm = (z + 1000) * mask  (DVE, reads PSUM; the -1000
                        # shift cancels exactly in exp(scale*(zm - max)))
                        zm = jz.tile([P, L], F32, tag="zm")
                        nc.vector.scalar_tensor_tensor(
                            zm[:], acc_s[:], thous[:], mstrip[:],
                            op0=ALU.add, op1=ALU.mult)
                        rmax = js.tile([P, 1], F32, tag="rmax")
                        nc.vector.reduce_max(rmax[:], zm[:], axis=AX.X)
                        bias = js.tile([P, 1], F32, tag="bias")
                        nc.vector.tensor_scalar_mul(bias[:], rmax[:], -SCALE)
                        sig = js.tile([P, 1], F32, tag="sig")
                        e = jz.tile([P, L], F32, tag="e")
                        nc.scalar.activation(e[:], zm[:], AF.Exp, bias=bias[:],
                                             scale=SCALE, accum_out=sig[:])
                        rinv = js.tile([P, 1], F32, tag="rinv")
                        nc.vector.reciprocal(rinv[:], sig[:])

                        outt = jo.tile([P, L], F32, tag="outt")
                        nc.vector.scalar_tensor_tensor(
                            outt[:], e[:], rinv[:], vj[:],
                            op0=ALU.mult, op1=ALU.mult)
                        nc.sync.dma_start(outT_d[jsl, :], outt[:])

    nc.compile()
    return nc


_NC = None


def _get_nc():
    global _NC
    if _NC is None:
        _NC = build()
    return _NC


def _make_in_maps(inputs):
    x = np.asarray(inputs["x"], dtype=np.float32)
    wq0 = np.asarray(inputs["wq"], dtype=np.float32)[0]
    wk0 = np.asarray(inputs["wk"], dtype=np.float32)[0]
    wv0 = np.ascontiguousarray(np.asarray(inputs["wv"], dtype=np.float32)[0])
    mask = np.asarray(inputs["mask"])
    wqT = np.ascontiguousarray(wq0.T)
    wkT = np.ascontiguousarray(wk0.T)
    eye = np.eye(P, dtype=np.float32)
    return [
        dict(
            xT=np.ascontiguousarray(x[b].T),
            xN=np.ascontiguousarray(x[b]),
            wqT=wqT, wkT=wkT, wv=wv0,
            maskT=np.ascontiguousarray(mask[b].T).astype(np.uint8),
            eye=eye,
        )
        for b in range(B)
    ]


def _gather(res):
    return np.stack(
        [res.results[b]["outT"].T for b in range(B)]).astype(np.float32)


def kernel(x, wq, wk, wv, mask):
    nc = _get_nc()
    in_maps = _make_in_maps(dict(x=x, wq=wq, wk=wk, wv=wv, mask=mask))
    res = run_bass_kernel_spmd(nc, in_maps, list(range(B)))
    return _gather(res)


if __name__ == "__main__":
    import tempfile
    from concourse.bass_utils import compile_bass_kernel
    nc = build()
    print("bass compile OK")
    with tempfile.TemporaryDirectory() as td:
        compile_bass_kernel(nc, td, "v6.neff")
    print("walrus compile OK")
